# revision 1
# baseline (speedup 1.0000x reference)
"""Trainium2 Bass kernel for nn_AST_GAT (gnn_message_passing).

Strategy
--------
The module's output is only ``out[index_map[root_ids]]`` — 64 rows of the
65536-row node state after 20 mean-aggregation SAGE iterations over the
combine-edge forest.  The dependency closure of those 64 rows through the 20
iterations is computed on the host (pure index manipulation: leaf maps, edge
pruning dynamics, reverse BFS).  All sparsity (segment-sums, per-iteration
pruning masks, mean normalization) is folded into small dense selection /
normalized-adjacency matrices, so the device does only dense matmuls.

Sharding: the 64 roots are split 8-per-core (graph-parallel); each core's
closure is independent, so there is no cross-core traffic.  The host gathers
the 8x[8,384] results into the full [64,384] output.

Device program per core (state kept feature-major so no transposes are ever
needed; the self-term and bias are folded into an augmented adjacency matrix
G' = [G; I; 1] applied to the stacked [u; v; bias] block):
  var_x   = vfT.T @ W_lin + b_lin x 1
  s_compT = var_x.T @ McompT         s_combT = subx.T @ McombT
  x0T     = Wl_c.T @ s_compT + Wl_b.T @ s_combT
            + (Wr_c+Wr_b).T @ subxT + (bl_c+bl_b) x 1
  iter i:  u = x @ Wl_lp ; v = x[:PP'] @ Wr_lp   (natural layout, N=384 muls)
           xT' = [u; v; bl_lp].T @ G2T_i          (G2T = [GT; I; 1])
  output: xT20[:, :n_roots] -> host transposes.
"""
import sys

sys.path.insert(0, "/opt/trn_rl_repo")

import numpy as np

N_ITERS = 20
N_CORES = 8
D = 384
KC = 3  # 128-chunks of D
P = 128

F32 = np.float32
USE_F32R = False  # flip to use fp32r (reduced-precision, faster) matmuls


# ======================================================================
# Host-side preprocessing (faithful numpy reimplementation of the
# reference's index semantics + dependency closure of the root rows).
# ======================================================================

def _append_unique(order, pos, items):
    for s in items:
        s = int(s)
        if s not in pos:
            pos[s] = len(order)
            order.append(s)


def graph_prep(E, root_ids, n_var, n_sub):
    src = np.asarray(E[0], dtype=np.int64)
    dst = np.asarray(E[1], dtype=np.int64)
    root_ids = np.asarray(root_ids, dtype=np.int64)
    N = n_var + n_sub

    leaf_mask = np.ones(N, dtype=bool)
    leaf_mask[dst] = False
    leaf_idx = np.cumsum(leaf_mask) - 1
    nleaf_idx = np.cumsum(~leaf_mask) - 1
    index_map = np.where(leaf_mask, leaf_idx, nleaf_idx)

    src_is_leaf = leaf_mask[src]
    src_loc = index_map[src]
    dst_loc = index_map[dst]
    src_s = np.clip(src_loc, 0, n_sub - 1)

    # simulate the pruning dynamics exactly as the reference scan does
    actives, dones = [], []
    active = ~src_is_leaf
    done = False
    for _ in range(N_ITERS):
        actives.append(active.copy())
        dones.append(done)
        idx = np.where(active, dst_loc, n_sub)
        valid = (idx >= 0) & (idx <= n_sub)  # jax scatter drops OOB
        is_tgt = np.zeros(n_sub + 1, dtype=bool)
        is_tgt[idx[valid]] = True
        active_new = active & is_tgt[src_s]
        if not done:
            active = active_new
        done = done or (not active.any())

    dmask = (dst_loc >= 0) & (dst_loc < n_sub)
    cnts = []
    for i in range(N_ITERS):
        if dones[i]:
            cnts.append(None)
        else:
            d = dst_loc[actives[i] & dmask]
            cnts.append(np.bincount(d, minlength=n_sub).astype(np.float64))

    return dict(
        src_is_leaf=src_is_leaf, src_loc=src_loc, dst_loc=dst_loc, src_s=src_s,
        actives=actives, dones=dones, cnts=cnts, dmask=dmask,
        j_roots=index_map[root_ids], n_sub=n_sub,
    )


def core_closure(g, roots):
    dst_loc, src_s = g["dst_loc"], g["src_s"]
    actives, dones, n_sub, dmask = g["actives"], g["dones"], g["n_sub"], g["dmask"]

    order, pos = [], {}
    _append_unique(order, pos, roots)
    lens = [0] * (N_ITERS + 1)
    lens[N_ITERS] = len(order)

    member = np.zeros(n_sub, dtype=bool)
    member[order] = True

    for i in range(N_ITERS - 1, -1, -1):
        if not dones[i]:
            e = actives[i] & dmask
            e[e] = member[dst_loc[e]]
            fresh = np.unique(src_s[e])
            fresh = fresh[~member[fresh]]
            _append_unique(order, pos, np.sort(fresh))
            member[fresh] = True
        lens[i] = len(order)

    posarr = np.full(n_sub, -1, dtype=np.int64)
    order_arr = np.array(order, dtype=np.int64)
    posarr[order_arr] = np.arange(len(order))

    in0 = dmask.copy()
    in0[dmask] = member[dst_loc[dmask]]

    comp_e = in0 & g["src_is_leaf"]
    leaves = g["src_loc"][comp_e]
    leaf_order, leaf_pos = [], {}
    _append_unique(leaf_order, leaf_pos, leaves)

    comb_e = in0 & ~g["src_is_leaf"]
    subs = np.clip(g["src_loc"][comb_e], 0, n_sub - 1)  # jax gather clamps
    sub_order, sub_pos = list(order), dict(pos)
    _append_unique(sub_order, sub_pos, subs)

    return dict(
        order=order_arr, pos=pos, posarr=posarr, lens=lens,
        comp_e=comp_e, leaves=leaves, leaf_order=np.array(leaf_order, np.int64),
        leaf_pos=leaf_pos,
        comb_e=comb_e, subs=subs, sub_order=np.array(sub_order, np.int64),
        sub_pos=sub_pos,
    )


def build_core_problem(g, cl, inputs, PPs, Lp, Pb):
    dst_loc, src_s = g["dst_loc"], g["src_s"]
    actives, dones, cnts, dmask = g["actives"], g["dones"], g["cnts"], g["dmask"]
    posarr, lens = cl["posarr"], cl["lens"]
    n0, PP0 = lens[0], PPs[0]
    n_sub = g["n_sub"]

    McompT = np.zeros((Lp, PP0), dtype=F32)
    if cl["leaves"].size:
        lcols = np.array([cl["leaf_pos"][int(s)] for s in cl["leaves"]], np.int64)
        np.add.at(McompT, (lcols, posarr[dst_loc[cl["comp_e"]]]), 1.0)

    McombT = np.zeros((Pb, PP0), dtype=F32)
    if cl["subs"].size:
        scols = np.array([cl["sub_pos"][int(s)] for s in cl["subs"]], np.int64)
        np.add.at(McombT, (scols, posarr[dst_loc[cl["comb_e"]]]), 1.0)

    GTs = []
    for i in range(N_ITERS):
        if dones[i]:
            GTs.append(None)
            continue
        nip1 = lens[i + 1]
        e = actives[i] & dmask
        sel = e.copy()
        p = posarr[dst_loc[e]]
        sel[e] = (p >= 0) & (p < nip1)
        GT = np.zeros((PPs[i], PPs[i + 1]), dtype=F32)
        if sel.any():
            rows = posarr[dst_loc[sel]]
            cols = posarr[src_s[sel]]
            vals = (1.0 / np.maximum(cnts[i][dst_loc[sel]], 1.0)).astype(F32)
            np.add.at(GT, (cols, rows), vals)
        GTs.append(GT)

    var_feats = np.asarray(inputs["var_feats"], dtype=F32)
    code_emb = np.asarray(inputs["code_emb"], dtype=F32)
    sids = np.clip(np.asarray(inputs["subcode_ids"], dtype=np.int64), 0,
                   code_emb.shape[0] - 1)

    vfT = np.zeros((D, Lp), dtype=F32)
    lo = cl["leaf_order"]
    if lo.size:
        vfT[:, : lo.size] = var_feats[lo].T
    subx = np.zeros((Pb, D), dtype=F32)
    so = cl["sub_order"]
    subx[: so.size] = code_emb[sids[so]]
    subxT_pref = np.zeros((D, PP0), dtype=F32)
    subxT_pref[:, :n0] = subx[:n0].T

    return dict(vfT=vfT, subx=subx, subxT=subxT_pref,
                mcompT=McompT, mcombT=McombT, GTs=GTs)


def preprocess(inputs):
    n_var = inputs["var_feats"].shape[0]
    n_sub = inputs["subcode_ids"].shape[0]
    root_ids = np.asarray(inputs["root_ids"], dtype=np.int64)
    B = root_ids.shape[0]
    assert B % N_CORES == 0
    per_core = B // N_CORES

    g = graph_prep(np.asarray(inputs["E"]), root_ids, n_var, n_sub)
    closures = [core_closure(g, g["j_roots"][c * per_core:(c + 1) * per_core])
                for c in range(N_CORES)]

    # pad to 32 so partition starts of the stacked [u; v; bias] segments are
    # 32-aligned (SBUF AP constraint) and no uninitialized gap rows exist
    PPs = [-(-max(cl["lens"][i] for cl in closures) // 32) * 32
           for i in range(N_ITERS + 1)]
    Lp = max(max(cl["leaf_order"].size for cl in closures), 1)
    Pb = max(max(cl["sub_order"].size for cl in closures), 1)

    probs = [build_core_problem(g, cl, inputs, PPs, Lp, Pb) for cl in closures]

    out_map = []
    for r in range(B):
        c = r // per_core
        j = int(g["j_roots"][r])
        out_map.append((c, closures[c]["pos"][j]))

    live = [i for i in range(N_ITERS) if not g["dones"][i]]
    return dict(probs=probs, PPs=PPs, Lp=Lp, Pb=Pb, out_map=out_map, live=live)


def stack_g2(GT, PPi, PPn):
    """Augmented adjacency: rows [0,PPi) = GT, [PPi,PPi+PPn) = identity
    (self term), plus a trailing ones row (bias) unless PPi+PPn is a
    multiple of 128 (then the bias is applied as a rank-1 matmul on the
    v psum instead, to avoid a 1-row extra partition chunk)."""
    bias_row = (PPi + PPn) % P != 0
    SP = PPi + PPn + (1 if bias_row else 0)
    G2 = np.zeros((SP, PPn), dtype=F32)
    G2[:PPi] = GT
    G2[PPi:PPi + PPn, :][np.arange(PPn), np.arange(PPn)] = 1.0
    if bias_row:
        G2[SP - 1, :] = 1.0
    return G2


# ======================================================================
# Device program
# ======================================================================

def _chunks(n):
    return [(s, min(P, n - s)) for s in range(0, n, P)]


def build_program(PPs, Lp, Pb, live):
    import concourse.mybir as mybir
    import concourse.tile as tile
    from concourse import bacc

    f32 = mybir.dt.float32
    mdt = mybir.dt.float32r if USE_F32R else f32
    PP0 = PPs[0]
    OUTW = max(PPs[N_ITERS], 1)

    nc = bacc.Bacc("TRN2", target_bir_lowering=False, debug=False,
                   num_devices=N_CORES)

    # ---- DRAM parameters ----
    wnames = ["wlin", "wlc", "wlb", "wrsum", "wllp", "wrlp"]
    wd = {n: nc.declare_dram_parameter(n, [D, D], mdt, isOutput=False)
          for n in wnames}
    bnames = ["blin", "blcb", "bllp"]
    bd = {n: nc.declare_dram_parameter(n, [1, D], mdt, isOutput=False)
          for n in bnames}
    ones_d = nc.declare_dram_parameter("ones", [1, 512], mdt, isOutput=False)
    vfT_d = nc.declare_dram_parameter("vfT", [D, Lp], mdt, isOutput=False)
    subx_d = nc.declare_dram_parameter("subx", [Pb, D], mdt, isOutput=False)
    subxT_d = nc.declare_dram_parameter("subxT", [D, PP0], mdt, isOutput=False)
    mcompT_d = nc.declare_dram_parameter("mcompT", [Lp, PP0], mdt, isOutput=False)
    mcombT_d = nc.declare_dram_parameter("mcombT", [Pb, PP0], mdt, isOutput=False)
    gt_d = {}
    for i in live:
        SP = PPs[i] + PPs[i + 1]
        if SP % P != 0:
            SP += 1
        gt_d[i] = nc.declare_dram_parameter(f"gt{i}", [SP, PPs[i + 1]], mdt,
                                            isOutput=False)
    out_d = nc.declare_dram_parameter("out", [KC, P, OUTW], f32, isOutput=True)

    with tile.TileContext(nc) as tc:
        with (
            tc.tile_pool(name="const", bufs=1) as const,
            tc.tile_pool(name="state", bufs=2) as state,
            tc.tile_pool(name="ps", bufs=4, space="PSUM") as ps,
        ):
            # ---- load constants, in first-use order ----
            # critical path first (varx needs wlin/blin/ones/vfT) on the
            # sync HWDGE queues; later-use bulk loads go via gpsimd SWDGE
            # so they don't delay the first matmuls.
            wt = {n: const.tile([P, KC * D], mdt, tag=n, name=n)
                  for n in wnames}
            bt = {n: const.tile([1, D], mdt, tag=n, name=n) for n in bnames}
            ones = const.tile([1, 512], mdt, tag="ones")

            def load_w(n, eng):
                for k in range(KC):
                    eng.dma_start(out=wt[n][:, k * D:(k + 1) * D],
                                  in_=wd[n][k * P:(k + 1) * P, :])

            nc.sync.dma_start(out=ones[:], in_=ones_d[:])
            # HAM warm-up: ~3.5us of dummy PE work issued while the
            # constant DMAs stream, so real matmuls start at 2.4 GHz
            for _ in range(8):
                wpt = ps.tile([P, 64], f32, tag="small_ps", name="warm")
                nc.tensor.matmul(out=wpt[:, :], lhsT=ones[:1, :P],
                                 rhs=ones[:1, :64], start=True, stop=True)
            load_w("wlin", nc.sync)
            nc.sync.dma_start(out=bt["blin"][:], in_=bd["blin"][:])
            vf_t = []
            for k in range(KC):
                t = const.tile([P, Lp], mdt, tag=f"vfT{k}")
                nc.sync.dma_start(out=t[:], in_=vfT_d[k * P:(k + 1) * P, :])
                vf_t.append(t)
            mcompT_t = []
            for ci, (s, sz) in enumerate(_chunks(Lp)):
                t = const.tile([P, PP0], mdt, tag=f"mcompT{ci}")
                nc.sync.dma_start(out=t[:sz, :], in_=mcompT_d[s:s + sz, :])
                mcompT_t.append(t)
            subx_t = []
            for ci, (s, sz) in enumerate(_chunks(Pb)):
                t = const.tile([P, D], mdt, tag=f"subx{ci}")
                nc.sync.dma_start(out=t[:sz, :], in_=subx_d[s:s + sz, :])
                subx_t.append(t)
            mcombT_t = []
            for ci, (s, sz) in enumerate(_chunks(Pb)):
                t = const.tile([P, PP0], mdt, tag=f"mcombT{ci}")
                nc.sync.dma_start(out=t[:sz, :], in_=mcombT_d[s:s + sz, :])
                mcombT_t.append(t)
            for n in ("wlc", "wlb", "wrsum"):
                load_w(n, nc.sync)
            nc.sync.dma_start(out=bt["blcb"][:], in_=bd["blcb"][:])
            subxT_t = []
            for k in range(KC):
                t = const.tile([P, PP0], mdt, tag=f"subxT{k}")
                nc.gpsimd.dma_start(out=t[:], in_=subxT_d[k * P:(k + 1) * P, :])
                subxT_t.append(t)
            load_w("wllp", nc.gpsimd)
            load_w("wrlp", nc.gpsimd)
            nc.gpsimd.dma_start(out=bt["bllp"][:], in_=bd["bllp"][:])
            gt_t = {}
            for i in live:
                SP = PPs[i] + PPs[i + 1]
                if SP % P != 0:
                    SP += 1
                lst = []
                for ci, (s, sz) in enumerate(_chunks(SP)):
                    t = const.tile([P, PPs[i + 1]], mdt, tag=f"gt{i}_{ci}")
                    nc.gpsimd.dma_start(out=t[:sz, :], in_=gt_d[i][s:s + sz, :])
                    lst.append(t)
                gt_t[i] = lst

            # ---- var_x = vfT.T @ W_lin + b_lin ----
            varx_t = []
            for ci, (s, sz) in enumerate(_chunks(Lp)):
                pt = ps.tile([P, D], f32, tag="big_ps")
                for k in range(KC):
                    nc.tensor.matmul(out=pt[:sz, :],
                                     lhsT=vf_t[k][:, s:s + sz],
                                     rhs=wt["wlin"][:, k * D:(k + 1) * D],
                                     start=(k == 0), stop=False)
                nc.tensor.matmul(out=pt[:sz, :], lhsT=ones[:1, :sz],
                                 rhs=bt["blin"][:1, :], start=False, stop=True)
                t = state.tile([P, D], mdt, tag=f"varx{ci}")
                nc.vector.tensor_copy(out=t[:sz, :], in_=pt[:sz, :])
                varx_t.append(t)

            # ---- s_compT / s_combT (feature-major) ----
            scompT_t, scombT_t = [], []
            lpch = _chunks(Lp)
            pbch = _chunks(Pb)
            for dk in range(KC):
                pt = ps.tile([P, PP0], f32, tag="small_ps")
                for ci, (s, sz) in enumerate(lpch):
                    nc.tensor.matmul(out=pt[:, :],
                                     lhsT=varx_t[ci][:sz, dk * P:(dk + 1) * P],
                                     rhs=mcompT_t[ci][:sz, :],
                                     start=(ci == 0), stop=(ci == len(lpch) - 1))
                t = state.tile([P, PP0], mdt, tag=f"scompT{dk}")
                nc.vector.tensor_copy(out=t[:], in_=pt[:])
                scompT_t.append(t)
            for dk in range(KC):
                pt = ps.tile([P, PP0], f32, tag="small_ps")
                for ci, (s, sz) in enumerate(pbch):
                    nc.tensor.matmul(out=pt[:, :],
                                     lhsT=subx_t[ci][:sz, dk * P:(dk + 1) * P],
                                     rhs=mcombT_t[ci][:sz, :],
                                     start=(ci == 0), stop=(ci == len(pbch) - 1))
                t = state.tile([P, PP0], mdt, tag=f"scombT{dk}")
                nc.vector.tensor_copy(out=t[:], in_=pt[:])
                scombT_t.append(t)

            # ---- x0T ----
            xT = []
            for dk in range(KC):
                pt = ps.tile([P, PP0], f32, tag="small_ps")
                for k in range(KC):
                    nc.tensor.matmul(out=pt[:, :],
                                     lhsT=wt["wlc"][:, k * D + dk * P:k * D + (dk + 1) * P],
                                     rhs=scompT_t[k][:, :],
                                     start=(k == 0), stop=False)
                for k in range(KC):
                    nc.tensor.matmul(out=pt[:, :],
                                     lhsT=wt["wlb"][:, k * D + dk * P:k * D + (dk + 1) * P],
                                     rhs=scombT_t[k][:, :],
                                     start=False, stop=False)
                for k in range(KC):
                    nc.tensor.matmul(out=pt[:, :],
                                     lhsT=wt["wrsum"][:, k * D + dk * P:k * D + (dk + 1) * P],
                                     rhs=subxT_t[k][:, :],
                                     start=False, stop=False)
                nc.tensor.matmul(out=pt[:, :],
                                 lhsT=bt["blcb"][:1, dk * P:(dk + 1) * P],
                                 rhs=ones[:1, :PP0], start=False, stop=True)
                t = state.tile([P, PP0], mdt, tag=f"xT{dk}_a")
                nc.vector.tensor_copy(out=t[:], in_=pt[:])
                xT.append(t)

            # ---- iterations ----
            for step, i in enumerate(live):
                PPi, PPn = PPs[i], PPs[i + 1]
                bias_row = (PPi + PPn) % P != 0
                SP = PPi + PPn + (1 if bias_row else 0)
                sch = _chunks(SP)
                us = [state.tile([P, D], mdt, tag=f"us{ci}", name=f"us{ci}_{step}")
                      for ci in range(len(sch))]

                def allowed(off):
                    # SBUF partition-start quadrant rule
                    return {0: P, 32: 32, 64: 64, 96: 32}[off]

                def copy_rows(gstart, pt, nrows):
                    r = 0
                    while r < nrows:
                        g0 = gstart + r
                        ci, off = g0 // P, g0 % P
                        take = min(allowed(off), allowed(r % P), nrows - r)
                        nc.vector.tensor_copy(out=us[ci][off:off + take, :],
                                              in_=pt[r:r + take, :])
                        r += take

                if SP <= P:
                    # single merged psum: u rows [0,PPi), v rows
                    # [PPi,PPi+PPn), bias row last -> one DVE copy
                    pt = ps.tile([P, D], f32, tag="big_ps")
                    for k in range(KC):
                        nc.tensor.matmul(out=pt[:PPi, :],
                                         lhsT=xT[k][:, :PPi],
                                         rhs=wt["wllp"][:, k * D:(k + 1) * D],
                                         start=(k == 0), stop=(k == KC - 1))
                    for k in range(KC):
                        nc.tensor.matmul(out=pt[PPi:PPi + PPn, :],
                                         lhsT=xT[k][:, :PPn],
                                         rhs=wt["wrlp"][:, k * D:(k + 1) * D],
                                         start=(k == 0),
                                         stop=(k == KC - 1 and bias_row))
                    bias_dve = False
                    if bias_row:
                        if (SP - 1) % P in (0, 32, 64):
                            nc.tensor.matmul(out=pt[SP - 1:SP, :],
                                             lhsT=ones[:1, :1],
                                             rhs=bt["bllp"][:1, :],
                                             start=True, stop=True)
                        else:  # matmul out base partition must be 0/32/64
                            bias_dve = True
                    else:
                        nc.tensor.matmul(out=pt[PPi:PPi + PPn, :],
                                         lhsT=ones[:1, :PPn],
                                         rhs=bt["bllp"][:1, :],
                                         start=False, stop=True)
                    ncp = SP - 1 if bias_dve else SP
                    nc.vector.tensor_copy(out=us[0][:ncp, :], in_=pt[:ncp, :])
                    if bias_dve:
                        nc.vector.tensor_copy(out=us[0][SP - 1:SP, :],
                                              in_=bt["bllp"][:1, :])
                else:
                    for (s, sz) in _chunks(PPi):
                        pt = ps.tile([P, D], f32, tag="big_ps")
                        for k in range(KC):
                            nc.tensor.matmul(out=pt[:sz, :],
                                             lhsT=xT[k][:, s:s + sz],
                                             rhs=wt["wllp"][:, k * D:(k + 1) * D],
                                             start=(k == 0), stop=(k == KC - 1))
                        copy_rows(s, pt, sz)
                    for (s, sz) in _chunks(PPn):
                        pt = ps.tile([P, D], f32, tag="big_ps")
                        for k in range(KC):
                            nc.tensor.matmul(out=pt[:sz, :],
                                             lhsT=xT[k][:, s:s + sz],
                                             rhs=wt["wrlp"][:, k * D:(k + 1) * D],
                                             start=(k == 0),
                                             stop=(k == KC - 1 and bias_row))
                        # without a stacked bias row, fold the bias into
                        # the v psum as a rank-1 matmul
                        if not bias_row:
                            nc.tensor.matmul(out=pt[:sz, :],
                                             lhsT=ones[:1, :sz],
                                             rhs=bt["bllp"][:1, :],
                                             start=False, stop=True)
                        copy_rows(PPi + s, pt, sz)
                    if bias_row:
                        gb = SP - 1
                        nc.vector.tensor_copy(
                            out=us[gb // P][gb % P:gb % P + 1, :],
                            in_=bt["bllp"][:1, :])

                ab = "ab"[step % 2]
                xTn = []
                for dk in range(KC):
                    pt = ps.tile([P, PPn], f32, tag="small_ps")
                    for ci, (s, sz) in enumerate(sch):
                        nc.tensor.matmul(out=pt[:, :],
                                         lhsT=us[ci][:sz, dk * P:(dk + 1) * P],
                                         rhs=gt_t[i][ci][:sz, :],
                                         start=(ci == 0), stop=(ci == len(sch) - 1))
                    t = state.tile([P, PPn], mdt, tag=f"xT{dk}_{ab}")
                    nc.vector.tensor_copy(out=t[:], in_=pt[:])
                    xTn.append(t)
                xT = xTn

            # ---- output ----
            for dk in range(KC):
                src = xT[dk][:, :OUTW]
                if USE_F32R:
                    src = src.bitcast(f32)
                nc.sync.dma_start(out=out_d[dk], in_=src)

    nc.compile()
    return nc


# ======================================================================
# Entry point
# ======================================================================

def kernel(**inputs) -> np.ndarray:
    out, _ = _run(inputs)
    return out


def _run(inputs, **spmd_kwargs):
    from concourse.bass_utils import run_bass_kernel_spmd

    pre = preprocess(inputs)
    PPs, Lp, Pb, live = pre["PPs"], pre["Lp"], pre["Pb"], pre["live"]

    nc = build_program(PPs, Lp, Pb, live)

    def f(a):
        return np.ascontiguousarray(np.asarray(a, F32))

    shared = {
        "wlin": f(inputs["W_lin"]), "wlc": f(inputs["Wl_c"]),
        "wlb": f(inputs["Wl_b"]),
        "wrsum": f(inputs["Wr_c"]) + f(inputs["Wr_b"]),
        "wllp": f(inputs["Wl_lp"]), "wrlp": f(inputs["Wr_lp"]),
        "blin": f(inputs["b_lin"]).reshape(1, D),
        "blcb": (f(inputs["bl_c"]) + f(inputs["bl_b"])).reshape(1, D),
        "bllp": f(inputs["bl_lp"]).reshape(1, D),
        "ones": np.ones((1, 512), dtype=F32),
    }

    in_maps = []
    for c in range(N_CORES):
        prob = pre["probs"][c]
        m = dict(shared)
        m["vfT"] = prob["vfT"]
        m["subx"] = prob["subx"]
        m["subxT"] = prob["subxT"]
        m["mcompT"] = prob["mcompT"]
        m["mcombT"] = prob["mcombT"]
        for i in live:
            m[f"gt{i}"] = stack_g2(prob["GTs"][i], PPs[i], PPs[i + 1])
        in_maps.append(m)

    res = run_bass_kernel_spmd(nc, in_maps, core_ids=list(range(N_CORES)),
                               **spmd_kwargs)

    B = len(pre["out_map"])
    OUTW = max(PPs[N_ITERS], 1)
    out = np.zeros((B, D), dtype=F32)
    for r, (c, row) in enumerate(pre["out_map"]):
        o = res.results[c]["out"].reshape(KC * P, OUTW)  # [384, OUTW]
        out[r] = o[:, row]
    return out, res



# revision 6
# speedup vs baseline: 1.6134x; 1.6134x over previous
"""Trainium2 Bass kernel for nn_AST_GAT (gnn_message_passing).

Strategy
--------
The module's output is only ``out[index_map[root_ids]]`` — 64 rows of the
65536-row node state after 20 mean-aggregation SAGE iterations over the
combine-edge forest.  The dependency closure of those 64 rows through the 20
iterations is computed on the host (pure index manipulation: leaf maps, edge
pruning dynamics, reverse BFS).  All sparsity (segment-sums, per-iteration
pruning masks, mean normalization) is folded into small dense selection /
normalized-adjacency matrices, so the device does only dense matmuls.

Sharding: the 64 roots are split 8-per-core (graph-parallel); each core's
closure is independent, so there is no cross-core traffic.  The host gathers
the 8x[8,384] results into the full [64,384] output.

Device program per core (state kept feature-major so no transposes are ever
needed; the self-term and bias are folded into an augmented adjacency matrix
G' = [G; I; 1] applied to the stacked [u; v; bias] block):
  var_x   = vfT.T @ W_lin + b_lin x 1
  s_compT = var_x.T @ McompT         s_combT = subx.T @ McombT
  x0T     = Wl_c.T @ s_compT + Wl_b.T @ s_combT
            + (Wr_c+Wr_b).T @ subxT + (bl_c+bl_b) x 1
  iter i:  u = x @ Wl_lp ; v = x[:PP'] @ Wr_lp   (natural layout, N=384 muls)
           xT' = [u; v; bl_lp].T @ G2T_i          (G2T = [GT; I; 1])
  output: xT20[:, :n_roots] -> host transposes.
"""
import sys

sys.path.insert(0, "/opt/trn_rl_repo")

import numpy as np

N_ITERS = 20
N_CORES = 8
D = 384
KC = 3  # 128-chunks of D
P = 128

F32 = np.float32
F16 = np.float16  # matmul operand dtype: fp16 streams 1 row/cycle on the PE
                  # (vs 4 for fp32) and halves DMA bytes; PSUM accumulates fp32


# ======================================================================
# Host-side preprocessing (faithful numpy reimplementation of the
# reference's index semantics + dependency closure of the root rows).
# ======================================================================

def _append_unique(order, pos, items):
    for s in items:
        s = int(s)
        if s not in pos:
            pos[s] = len(order)
            order.append(s)


def graph_prep(E, root_ids, n_var, n_sub):
    src = np.asarray(E[0], dtype=np.int64)
    dst = np.asarray(E[1], dtype=np.int64)
    root_ids = np.asarray(root_ids, dtype=np.int64)
    N = n_var + n_sub

    leaf_mask = np.ones(N, dtype=bool)
    leaf_mask[dst] = False
    leaf_idx = np.cumsum(leaf_mask) - 1
    nleaf_idx = np.cumsum(~leaf_mask) - 1
    index_map = np.where(leaf_mask, leaf_idx, nleaf_idx)

    src_is_leaf = leaf_mask[src]
    src_loc = index_map[src]
    dst_loc = index_map[dst]
    src_s = np.clip(src_loc, 0, n_sub - 1)

    # simulate the pruning dynamics exactly as the reference scan does
    actives, dones = [], []
    active = ~src_is_leaf
    done = False
    for _ in range(N_ITERS):
        actives.append(active.copy())
        dones.append(done)
        idx = np.where(active, dst_loc, n_sub)
        valid = (idx >= 0) & (idx <= n_sub)  # jax scatter drops OOB
        is_tgt = np.zeros(n_sub + 1, dtype=bool)
        is_tgt[idx[valid]] = True
        active_new = active & is_tgt[src_s]
        if not done:
            active = active_new
        done = done or (not active.any())

    dmask = (dst_loc >= 0) & (dst_loc < n_sub)
    cnts = []
    for i in range(N_ITERS):
        if dones[i]:
            cnts.append(None)
        else:
            d = dst_loc[actives[i] & dmask]
            cnts.append(np.bincount(d, minlength=n_sub).astype(np.float64))

    return dict(
        src_is_leaf=src_is_leaf, src_loc=src_loc, dst_loc=dst_loc, src_s=src_s,
        actives=actives, dones=dones, cnts=cnts, dmask=dmask,
        j_roots=index_map[root_ids], n_sub=n_sub,
    )


def core_closure(g, roots):
    dst_loc, src_s = g["dst_loc"], g["src_s"]
    actives, dones, n_sub, dmask = g["actives"], g["dones"], g["n_sub"], g["dmask"]

    order, pos = [], {}
    _append_unique(order, pos, roots)
    lens = [0] * (N_ITERS + 1)
    lens[N_ITERS] = len(order)

    member = np.zeros(n_sub, dtype=bool)
    member[order] = True

    for i in range(N_ITERS - 1, -1, -1):
        if not dones[i]:
            e = actives[i] & dmask
            e[e] = member[dst_loc[e]]
            fresh = np.unique(src_s[e])
            fresh = fresh[~member[fresh]]
            _append_unique(order, pos, np.sort(fresh))
            member[fresh] = True
        lens[i] = len(order)

    posarr = np.full(n_sub, -1, dtype=np.int64)
    order_arr = np.array(order, dtype=np.int64)
    posarr[order_arr] = np.arange(len(order))

    in0 = dmask.copy()
    in0[dmask] = member[dst_loc[dmask]]

    comp_e = in0 & g["src_is_leaf"]
    leaves = g["src_loc"][comp_e]
    leaf_order, leaf_pos = [], {}
    _append_unique(leaf_order, leaf_pos, leaves)

    comb_e = in0 & ~g["src_is_leaf"]
    subs = np.clip(g["src_loc"][comb_e], 0, n_sub - 1)  # jax gather clamps
    sub_order, sub_pos = list(order), dict(pos)
    _append_unique(sub_order, sub_pos, subs)

    return dict(
        order=order_arr, pos=pos, posarr=posarr, lens=lens,
        comp_e=comp_e, leaves=leaves, leaf_order=np.array(leaf_order, np.int64),
        leaf_pos=leaf_pos,
        comb_e=comb_e, subs=subs, sub_order=np.array(sub_order, np.int64),
        sub_pos=sub_pos,
    )


def build_core_problem(g, cl, inputs, PPs, Lp, Pb):
    dst_loc, src_s = g["dst_loc"], g["src_s"]
    actives, dones, cnts, dmask = g["actives"], g["dones"], g["cnts"], g["dmask"]
    posarr, lens = cl["posarr"], cl["lens"]
    n0, PP0 = lens[0], PPs[0]
    n_sub = g["n_sub"]

    McompT = np.zeros((Lp, PP0), dtype=F32)
    if cl["leaves"].size:
        lcols = np.array([cl["leaf_pos"][int(s)] for s in cl["leaves"]], np.int64)
        np.add.at(McompT, (lcols, posarr[dst_loc[cl["comp_e"]]]), 1.0)

    McombT = np.zeros((Pb, PP0), dtype=F32)
    if cl["subs"].size:
        scols = np.array([cl["sub_pos"][int(s)] for s in cl["subs"]], np.int64)
        np.add.at(McombT, (scols, posarr[dst_loc[cl["comb_e"]]]), 1.0)

    GTs = []
    for i in range(N_ITERS):
        if dones[i]:
            GTs.append(None)
            continue
        nip1 = lens[i + 1]
        e = actives[i] & dmask
        sel = e.copy()
        p = posarr[dst_loc[e]]
        sel[e] = (p >= 0) & (p < nip1)
        GT = np.zeros((PPs[i], PPs[i + 1]), dtype=F32)
        if sel.any():
            rows = posarr[dst_loc[sel]]
            cols = posarr[src_s[sel]]
            vals = (1.0 / np.maximum(cnts[i][dst_loc[sel]], 1.0)).astype(F32)
            np.add.at(GT, (cols, rows), vals)
        GTs.append(GT)

    var_feats = np.asarray(inputs["var_feats"], dtype=F32)
    code_emb = np.asarray(inputs["code_emb"], dtype=F32)
    sids = np.clip(np.asarray(inputs["subcode_ids"], dtype=np.int64), 0,
                   code_emb.shape[0] - 1)

    vfT = np.zeros((D, Lp), dtype=F32)
    lo = cl["leaf_order"]
    if lo.size:
        vfT[:, : lo.size] = var_feats[lo].T
    subx = np.zeros((Pb, D), dtype=F32)
    so = cl["sub_order"]
    subx[: so.size] = code_emb[sids[so]]
    subxT_pref = np.zeros((D, PP0), dtype=F32)
    subxT_pref[:, :n0] = subx[:n0].T

    return dict(vfT=vfT, subx=subx, subxT=subxT_pref,
                mcompT=McompT, mcombT=McombT, GTs=GTs)


def preprocess(inputs):
    n_var = inputs["var_feats"].shape[0]
    n_sub = inputs["subcode_ids"].shape[0]
    root_ids = np.asarray(inputs["root_ids"], dtype=np.int64)
    B = root_ids.shape[0]
    assert B % N_CORES == 0
    per_core = B // N_CORES

    g = graph_prep(np.asarray(inputs["E"]), root_ids, n_var, n_sub)
    closures = [core_closure(g, g["j_roots"][c * per_core:(c + 1) * per_core])
                for c in range(N_CORES)]

    # pad to 32 so partition starts of the stacked [u; v; bias] segments are
    # 32-aligned (SBUF AP constraint) and no uninitialized gap rows exist
    PPs = [-(-max(cl["lens"][i] for cl in closures) // 32) * 32
           for i in range(N_ITERS + 1)]
    Lp = max(max(cl["leaf_order"].size for cl in closures), 1)
    Pb = max(max(cl["sub_order"].size for cl in closures), 1)

    probs = [build_core_problem(g, cl, inputs, PPs, Lp, Pb) for cl in closures]

    out_map = []
    for r in range(B):
        c = r // per_core
        j = int(g["j_roots"][r])
        out_map.append((c, closures[c]["pos"][j]))

    live = [i for i in range(N_ITERS) if not g["dones"][i]]
    return dict(probs=probs, PPs=PPs, Lp=Lp, Pb=Pb, out_map=out_map, live=live)


def stack_g2(GT, PPi, PPn):
    """Augmented adjacency: rows [0,PPi) = GT, [PPi,PPi+PPn) = identity
    (self term), plus a trailing ones row (bias) unless PPi+PPn is a
    multiple of 128 (then the bias is applied as a rank-1 matmul on the
    v psum instead, to avoid a 1-row extra partition chunk)."""
    bias_row = (PPi + PPn) % P != 0
    SP = PPi + PPn + (1 if bias_row else 0)
    G2 = np.zeros((SP, PPn), dtype=F32)
    G2[:PPi] = GT
    G2[PPi:PPi + PPn, :][np.arange(PPn), np.arange(PPn)] = 1.0
    if bias_row:
        G2[SP - 1, :] = 1.0
    return G2


# ======================================================================
# Device program
# ======================================================================

def _chunks(n):
    return [(s, min(P, n - s)) for s in range(0, n, P)]


def build_program(PPs, Lp, Pb, live):
    import concourse.mybir as mybir
    import concourse.tile as tile
    from concourse import bacc

    f32 = mybir.dt.float32
    mdt = mybir.dt.float16
    PP0 = PPs[0]
    OUTW = max(PPs[N_ITERS], 1)

    nc = bacc.Bacc("TRN2", target_bir_lowering=False, debug=False,
                   num_devices=N_CORES)

    # ---- DRAM parameters ----
    wnames = ["wlin", "wlc", "wlb", "wrsum", "wllp", "wrlp"]
    wd = {n: nc.declare_dram_parameter(n, [D, D], mdt, isOutput=False)
          for n in wnames}
    bnames = ["blin", "blcb", "bllp"]
    bd = {n: nc.declare_dram_parameter(n, [1, D], mdt, isOutput=False)
          for n in bnames}
    ones_d = nc.declare_dram_parameter("ones", [1, 512], mdt, isOutput=False)
    vfT_d = nc.declare_dram_parameter("vfT", [D, Lp], mdt, isOutput=False)
    subx_d = nc.declare_dram_parameter("subx", [Pb, D], mdt, isOutput=False)
    subxT_d = nc.declare_dram_parameter("subxT", [D, PP0], mdt, isOutput=False)
    mcompT_d = nc.declare_dram_parameter("mcompT", [Lp, PP0], mdt, isOutput=False)
    mcombT_d = nc.declare_dram_parameter("mcombT", [Pb, PP0], mdt, isOutput=False)
    gt_d = {}
    for i in live:
        SP = PPs[i] + PPs[i + 1]
        if SP % P != 0:
            SP += 1
        gt_d[i] = nc.declare_dram_parameter(f"gt{i}", [SP, PPs[i + 1]], mdt,
                                            isOutput=False)
    out_d = nc.declare_dram_parameter("out", [KC, P, OUTW], f32, isOutput=True)

    with tile.TileContext(nc) as tc:
        with (
            tc.tile_pool(name="const", bufs=1) as const,
            tc.tile_pool(name="state", bufs=2) as state,
            tc.tile_pool(name="ps", bufs=4, space="PSUM") as ps,
        ):
            # ---- load constants, in first-use order ----
            # critical path first (varx needs wlin/blin/ones/vfT) on the
            # sync HWDGE queues; later-use bulk loads go via gpsimd SWDGE
            # so they don't delay the first matmuls.
            wt = {n: const.tile([P, KC * D], mdt, tag=n, name=n)
                  for n in wnames}
            bt = {n: const.tile([1, D], mdt, tag=n, name=n) for n in bnames}
            ones = const.tile([1, 512], mdt, tag="ones")

            def load_w(n, eng):
                for k in range(KC):
                    eng.dma_start(out=wt[n][:, k * D:(k + 1) * D],
                                  in_=wd[n][k * P:(k + 1) * P, :])

            nc.sync.dma_start(out=ones[:], in_=ones_d[:])
            # HAM warm-up: ~3.5us of dummy PE work issued while the
            # constant DMAs stream, so real matmuls start at 2.4 GHz
            for _ in range(8):
                wpt = ps.tile([P, 64], f32, tag="small_ps", name="warm")
                nc.tensor.matmul(out=wpt[:, :], lhsT=ones[:1, :P],
                                 rhs=ones[:1, :64], start=True, stop=True)
            load_w("wlin", nc.sync)
            nc.sync.dma_start(out=bt["blin"][:], in_=bd["blin"][:])
            vf_t = []
            for k in range(KC):
                t = const.tile([P, Lp], mdt, tag=f"vfT{k}")
                nc.sync.dma_start(out=t[:], in_=vfT_d[k * P:(k + 1) * P, :])
                vf_t.append(t)
            mcompT_t = []
            for ci, (s, sz) in enumerate(_chunks(Lp)):
                t = const.tile([P, PP0], mdt, tag=f"mcompT{ci}")
                nc.sync.dma_start(out=t[:sz, :], in_=mcompT_d[s:s + sz, :])
                mcompT_t.append(t)
            subx_t = []
            for ci, (s, sz) in enumerate(_chunks(Pb)):
                t = const.tile([P, D], mdt, tag=f"subx{ci}")
                nc.sync.dma_start(out=t[:sz, :], in_=subx_d[s:s + sz, :])
                subx_t.append(t)
            mcombT_t = []
            for ci, (s, sz) in enumerate(_chunks(Pb)):
                t = const.tile([P, PP0], mdt, tag=f"mcombT{ci}")
                nc.sync.dma_start(out=t[:sz, :], in_=mcombT_d[s:s + sz, :])
                mcombT_t.append(t)
            for n in ("wlc", "wlb", "wrsum"):
                load_w(n, nc.sync)
            nc.sync.dma_start(out=bt["blcb"][:], in_=bd["blcb"][:])
            subxT_t = []
            for k in range(KC):
                t = const.tile([P, PP0], mdt, tag=f"subxT{k}")
                nc.gpsimd.dma_start(out=t[:], in_=subxT_d[k * P:(k + 1) * P, :])
                subxT_t.append(t)
            load_w("wllp", nc.gpsimd)
            load_w("wrlp", nc.gpsimd)
            nc.gpsimd.dma_start(out=bt["bllp"][:], in_=bd["bllp"][:])
            gt_t = {}
            for i in live:
                SP = PPs[i] + PPs[i + 1]
                if SP % P != 0:
                    SP += 1
                lst = []
                for ci, (s, sz) in enumerate(_chunks(SP)):
                    t = const.tile([P, PPs[i + 1]], mdt, tag=f"gt{i}_{ci}")
                    nc.gpsimd.dma_start(out=t[:sz, :], in_=gt_d[i][s:s + sz, :])
                    lst.append(t)
                gt_t[i] = lst

            # ---- var_x = vfT.T @ W_lin + b_lin ----
            varx_t = []
            for ci, (s, sz) in enumerate(_chunks(Lp)):
                pt = ps.tile([P, D], f32, tag="big_ps")
                for k in range(KC):
                    nc.tensor.matmul(out=pt[:sz, :],
                                     lhsT=vf_t[k][:, s:s + sz],
                                     rhs=wt["wlin"][:, k * D:(k + 1) * D],
                                     start=(k == 0), stop=False)
                nc.tensor.matmul(out=pt[:sz, :], lhsT=ones[:1, :sz],
                                 rhs=bt["blin"][:1, :], start=False, stop=True)
                t = state.tile([P, D], mdt, tag=f"varx{ci}")
                nc.vector.tensor_copy(out=t[:sz, :], in_=pt[:sz, :])
                varx_t.append(t)

            # ---- s_compT / s_combT (feature-major) ----
            scompT_t, scombT_t = [], []
            lpch = _chunks(Lp)
            pbch = _chunks(Pb)
            for dk in range(KC):
                pt = ps.tile([P, PP0], f32, tag="small_ps")
                for ci, (s, sz) in enumerate(lpch):
                    nc.tensor.matmul(out=pt[:, :],
                                     lhsT=varx_t[ci][:sz, dk * P:(dk + 1) * P],
                                     rhs=mcompT_t[ci][:sz, :],
                                     start=(ci == 0), stop=(ci == len(lpch) - 1))
                t = state.tile([P, PP0], mdt, tag=f"scompT{dk}")
                nc.vector.tensor_copy(out=t[:], in_=pt[:])
                scompT_t.append(t)
            for dk in range(KC):
                pt = ps.tile([P, PP0], f32, tag="small_ps")
                for ci, (s, sz) in enumerate(pbch):
                    nc.tensor.matmul(out=pt[:, :],
                                     lhsT=subx_t[ci][:sz, dk * P:(dk + 1) * P],
                                     rhs=mcombT_t[ci][:sz, :],
                                     start=(ci == 0), stop=(ci == len(pbch) - 1))
                t = state.tile([P, PP0], mdt, tag=f"scombT{dk}")
                nc.vector.tensor_copy(out=t[:], in_=pt[:])
                scombT_t.append(t)

            # ---- x0T ----
            xT = []
            for dk in range(KC):
                pt = ps.tile([P, PP0], f32, tag="small_ps")
                for k in range(KC):
                    nc.tensor.matmul(out=pt[:, :],
                                     lhsT=wt["wlc"][:, k * D + dk * P:k * D + (dk + 1) * P],
                                     rhs=scompT_t[k][:, :],
                                     start=(k == 0), stop=False)
                for k in range(KC):
                    nc.tensor.matmul(out=pt[:, :],
                                     lhsT=wt["wlb"][:, k * D + dk * P:k * D + (dk + 1) * P],
                                     rhs=scombT_t[k][:, :],
                                     start=False, stop=False)
                for k in range(KC):
                    nc.tensor.matmul(out=pt[:, :],
                                     lhsT=wt["wrsum"][:, k * D + dk * P:k * D + (dk + 1) * P],
                                     rhs=subxT_t[k][:, :],
                                     start=False, stop=False)
                nc.tensor.matmul(out=pt[:, :],
                                 lhsT=bt["blcb"][:1, dk * P:(dk + 1) * P],
                                 rhs=ones[:1, :PP0], start=False, stop=True)
                t = state.tile([P, PP0], mdt, tag=f"xT{dk}_a")
                nc.vector.tensor_copy(out=t[:], in_=pt[:])
                xT.append(t)

            # ---- iterations ----
            for step, i in enumerate(live):
                PPi, PPn = PPs[i], PPs[i + 1]
                bias_row = (PPi + PPn) % P != 0
                SP = PPi + PPn + (1 if bias_row else 0)
                sch = _chunks(SP)
                us = [state.tile([P, D], mdt, tag=f"us{ci}", name=f"us{ci}_{step}")
                      for ci in range(len(sch))]

                def allowed(off):
                    # SBUF partition-start quadrant rule
                    return {0: P, 32: 32, 64: 64, 96: 32}[off]

                def copy_rows(gstart, pt, nrows):
                    r = 0
                    while r < nrows:
                        g0 = gstart + r
                        ci, off = g0 // P, g0 % P
                        take = min(allowed(off), allowed(r % P), nrows - r)
                        nc.vector.tensor_copy(out=us[ci][off:off + take, :],
                                              in_=pt[r:r + take, :])
                        r += take

                if SP <= P:
                    # single merged psum: u rows [0,PPi), v rows
                    # [PPi,PPi+PPn), bias row last -> one DVE copy
                    pt = ps.tile([P, D], f32, tag="big_ps")
                    for k in range(KC):
                        nc.tensor.matmul(out=pt[:PPi, :],
                                         lhsT=xT[k][:, :PPi],
                                         rhs=wt["wllp"][:, k * D:(k + 1) * D],
                                         start=(k == 0), stop=(k == KC - 1))
                    for k in range(KC):
                        nc.tensor.matmul(out=pt[PPi:PPi + PPn, :],
                                         lhsT=xT[k][:, :PPn],
                                         rhs=wt["wrlp"][:, k * D:(k + 1) * D],
                                         start=(k == 0),
                                         stop=(k == KC - 1 and bias_row))
                    bias_dve = False
                    if bias_row:
                        if (SP - 1) % P in (0, 32, 64):
                            nc.tensor.matmul(out=pt[SP - 1:SP, :],
                                             lhsT=ones[:1, :1],
                                             rhs=bt["bllp"][:1, :],
                                             start=True, stop=True)
                        else:  # matmul out base partition must be 0/32/64
                            bias_dve = True
                    else:
                        nc.tensor.matmul(out=pt[PPi:PPi + PPn, :],
                                         lhsT=ones[:1, :PPn],
                                         rhs=bt["bllp"][:1, :],
                                         start=False, stop=True)
                    ncp = SP - 1 if bias_dve else SP
                    nc.vector.tensor_copy(out=us[0][:ncp, :], in_=pt[:ncp, :])
                    if bias_dve:
                        nc.vector.tensor_copy(out=us[0][SP - 1:SP, :],
                                              in_=bt["bllp"][:1, :])
                else:
                    for (s, sz) in _chunks(PPi):
                        pt = ps.tile([P, D], f32, tag="big_ps")
                        for k in range(KC):
                            nc.tensor.matmul(out=pt[:sz, :],
                                             lhsT=xT[k][:, s:s + sz],
                                             rhs=wt["wllp"][:, k * D:(k + 1) * D],
                                             start=(k == 0), stop=(k == KC - 1))
                        copy_rows(s, pt, sz)
                    for (s, sz) in _chunks(PPn):
                        pt = ps.tile([P, D], f32, tag="big_ps")
                        for k in range(KC):
                            nc.tensor.matmul(out=pt[:sz, :],
                                             lhsT=xT[k][:, s:s + sz],
                                             rhs=wt["wrlp"][:, k * D:(k + 1) * D],
                                             start=(k == 0),
                                             stop=(k == KC - 1 and bias_row))
                        # without a stacked bias row, fold the bias into
                        # the v psum as a rank-1 matmul
                        if not bias_row:
                            nc.tensor.matmul(out=pt[:sz, :],
                                             lhsT=ones[:1, :sz],
                                             rhs=bt["bllp"][:1, :],
                                             start=False, stop=True)
                        copy_rows(PPi + s, pt, sz)
                    if bias_row:
                        gb = SP - 1
                        nc.vector.tensor_copy(
                            out=us[gb // P][gb % P:gb % P + 1, :],
                            in_=bt["bllp"][:1, :])

                ab = "ab"[step % 2]
                # final step's tiles feed the f32 output DMA directly
                odt = f32 if step == len(live) - 1 else mdt
                xTn = []
                for dk in range(KC):
                    pt = ps.tile([P, PPn], f32, tag="small_ps")
                    for ci, (s, sz) in enumerate(sch):
                        nc.tensor.matmul(out=pt[:, :],
                                         lhsT=us[ci][:sz, dk * P:(dk + 1) * P],
                                         rhs=gt_t[i][ci][:sz, :],
                                         start=(ci == 0), stop=(ci == len(sch) - 1))
                    t = state.tile([P, PPn], odt, tag=f"xT{dk}_{ab}")
                    nc.vector.tensor_copy(out=t[:], in_=pt[:])
                    xTn.append(t)
                xT = xTn

            # ---- output ----
            for dk in range(KC):
                nc.sync.dma_start(out=out_d[dk], in_=xT[dk][:, :OUTW])

    nc.compile()
    return nc


# ======================================================================
# Entry point
# ======================================================================

def kernel(**inputs) -> np.ndarray:
    out, _ = _run(inputs)
    return out


def _run(inputs, **spmd_kwargs):
    from concourse.bass_utils import run_bass_kernel_spmd

    pre = preprocess(inputs)
    PPs, Lp, Pb, live = pre["PPs"], pre["Lp"], pre["Pb"], pre["live"]

    nc = build_program(PPs, Lp, Pb, live)

    def f(a):
        return np.ascontiguousarray(np.asarray(a, F32))

    def h(a):
        return np.ascontiguousarray(np.asarray(a, F16))

    shared = {
        "wlin": h(inputs["W_lin"]), "wlc": h(inputs["Wl_c"]),
        "wlb": h(inputs["Wl_b"]),
        "wrsum": h(f(inputs["Wr_c"]) + f(inputs["Wr_b"])),
        "wllp": h(inputs["Wl_lp"]), "wrlp": h(inputs["Wr_lp"]),
        "blin": h(inputs["b_lin"]).reshape(1, D),
        "blcb": h(f(inputs["bl_c"]) + f(inputs["bl_b"])).reshape(1, D),
        "bllp": h(inputs["bl_lp"]).reshape(1, D),
        "ones": np.ones((1, 512), dtype=F16),
    }

    in_maps = []
    for c in range(N_CORES):
        prob = pre["probs"][c]
        m = dict(shared)
        m["vfT"] = h(prob["vfT"])
        m["subx"] = h(prob["subx"])
        m["subxT"] = h(prob["subxT"])
        m["mcompT"] = h(prob["mcompT"])
        m["mcombT"] = h(prob["mcombT"])
        for i in live:
            m[f"gt{i}"] = h(stack_g2(prob["GTs"][i], PPs[i], PPs[i + 1]))
        in_maps.append(m)

    res = run_bass_kernel_spmd(nc, in_maps, core_ids=list(range(N_CORES)),
                               **spmd_kwargs)

    B = len(pre["out_map"])
    OUTW = max(PPs[N_ITERS], 1)
    out = np.zeros((B, D), dtype=F32)
    for r, (c, row) in enumerate(pre["out_map"]):
        o = res.results[c]["out"].reshape(KC * P, OUTW)  # [384, OUTW]
        out[r] = o[:, row]
    return out, res

